# revision 23
# baseline (speedup 1.0000x reference)
"""Trainium2 Bass kernel for nn_Encoder_71313636983306 (pillar scatter encoder).

Computes, for each (batch, frame) pair:
    emb = relu(BN(Linear(pcl))) * mask          # [N, 64] point embeddings
    grid = scatter_add(emb, cell_idx)           # [64, 640*640]
and returns the 4 grids stacked as [B*2, 64, 640, 640] (f32).

Sharding: 8 cores = 4 (batch, frame) pairs x 2 grid halves. Each core
processes the (unmasked) points of its pair that land in its half of the
640x640 grid and writes a dense [64, 204800] f32 half-grid.

Device algorithm (per core): the half-grid is covered by 400 tasks; task j
owns cells [256j, 256j+256) ("A") and [102400+256j, +256) ("B"). The host
packs each task's points (<=128, guaranteed by the ~0.12 pts/cell density)
into 128 "slots". All PE matmuls use bf16 hi/lo splits (full fp32 PE passes
are 4x slower and fp32 weight loads get no fast-weight-load). Per task:
  1. pointnet: 3 accumulating bf16 matmuls (xh@Wh + xl@Wh + xh@Wl, residual
     ~2^-18) -> PSUM [128slots, 128]: emb placed in columns 0:64 for A-points
     / 64:128 for B-points, bias folded in via a constant-1 coordinate row.
  2. relu: ScalarE PSUM->SBUF twice (bf16 "hi" + f32), VectorE subtract
     gives the bf16 "lo" residual.
  3. one-hot M[128slots, 256] bf16 = is_equal(iota, local_cell_idx) (DVE).
  4. grid matmul emb^T @ M as two accumulating bf16 matmuls (hi + lo)
     -> PSUM [128, 256] = the task's 512 output cells.
  5. copy PSUM -> SBUF staging (DVE/ACT alternating); every 16 tasks one
     2 MB DMA writes the staging buffer to the output grid in HBM.
"""
import numpy as np
import ml_dtypes

BF16 = ml_dtypes.bfloat16

# ---------------------------------------------------------------- constants
B = 2
N_PTS = 100000
D = 64
N_PX = N_PY = 640
P_CELLS = N_PX * N_PY          # 409600
HALF_CELLS = P_CELLS // 2      # 204800 cells per core
WIN = 512                      # cells per task
T = HALF_CELLS // WIN          # 400 tasks per core
NSLOT = 128                    # point slots per task
CHUNK_T = 40                   # tasks per resident XT chunk (multiple of 4)
FLUSH_T = 16                   # tasks per output DMA flush
BN_EPS = 1e-5
N_CORES = 8
GRID_MODE = "hilo"             # "hilo": bf16 hi+lo grid matmuls; "f16": single f16
COPY_PATTERN = "vs"            # stage-copy engines by pair index (v=DVE, s=ACT)
M_PATTERN = "vg"                # one-hot build engines by task (v=DVE, g=GPSIMD)
LDW_OPT = False                # rewrite walrus --enable-ldw-opt flag (incompatible)

_cached = {}


# ---------------------------------------------------------------- device code
def _build_kernel():
    from contextlib import ExitStack
    import concourse.tile as tile
    from concourse import bacc, mybir

    f32 = mybir.dt.float32
    bf16 = mybir.dt.bfloat16

    if LDW_OPT:
        import concourse.bass_utils as _bu
        if not getattr(_bu, "_ldw_patched", False):
            _orig = _bu.run_command

            def _patched(argv, **kw):
                argv = ["--enable-ldw-opt=true" if a == "--enable-ldw-opt=false"
                        else a for a in argv]
                return _orig(argv, **kw)

            _bu.run_command = _patched
            _bu._ldw_patched = True

    nc = bacc.Bacc("TRN2", target_bir_lowering=False, debug=False,
                   num_devices=N_CORES)

    i16 = mybir.dt.int16
    f16 = mybir.dt.float16
    emb_dt = f16 if GRID_MODE == "f16" else bf16

    xt24 = nc.dram_tensor("xt24", [24, T * NSLOT], bf16,
                          kind="ExternalInput").ap()
    scat = nc.dram_tensor("scat", [NSLOT, 2 * T], i16,
                          kind="ExternalInput").ap()
    idxc = nc.dram_tensor("idxc", [NSLOT, T], f32, kind="ExternalInput").ap()
    iota = nc.dram_tensor("iota", [NSLOT, 256], emb_dt,
                          kind="ExternalInput").ap()
    w24 = nc.dram_tensor("w24", [24, NSLOT], bf16, kind="ExternalInput").ap()
    # Output keeps the staging layout: row p = 64*h + d holds cells
    # [102400*h + 256*j, +256) of task j; the host deinterleaves the halves.
    grid = nc.dram_tensor("grid", [2 * D, T * 256], f32,
                          kind="ExternalOutput").ap()

    with tile.TileContext(nc) as tc:
        with ExitStack() as ctx:
            consts = ctx.enter_context(tc.tile_pool(name="consts", bufs=1))
            xt_pool = ctx.enter_context(tc.tile_pool(name="xtc", bufs=3))
            emb_pool = ctx.enter_context(tc.tile_pool(name="emb", bufs=4))
            m_pool = ctx.enter_context(tc.tile_pool(name="m", bufs=12))
            stage_pool = ctx.enter_context(tc.tile_pool(name="stage", bufs=3))
            pn_psum = ctx.enter_context(
                tc.tile_pool(name="pnps", bufs=2, space="PSUM"))
            gr_psum = ctx.enter_context(
                tc.tile_pool(name="grps", bufs=3, space="PSUM"))

            w24_t = consts.tile([24, NSLOT], bf16)
            nc.sync.dma_start(w24_t[:], w24[:])
            scat_t = consts.tile([NSLOT, 2 * T], i16)
            nc.sync.dma_start(scat_t[:], scat[:])
            idxc_t = consts.tile([NSLOT, T], f32)
            nc.sync.dma_start(idxc_t[:], idxc[:])
            iota_t = consts.tile([NSLOT, 256], emb_dt)
            nc.sync.dma_start(iota_t[:], iota[:])
            ones2 = consts.tile([NSLOT, 2], emb_dt)
            nc.gpsimd.memset(ones2[:], 1.0)

            xc = None
            stage = None
            gr = None
            for g in range(T // 8):            # octet of 8 tasks
                j0 = 8 * g
                if j0 % CHUNK_T == 0:
                    xc = xt_pool.tile([24, CHUNK_T * NSLOT], bf16)
                    nc.sync.dma_start(
                        xc[:], xt24[:, j0 * NSLOT:(j0 + CHUNK_T) * NSLOT])
                if j0 % FLUSH_T == 0:
                    stage = stage_pool.tile([NSLOT, FLUSH_T * 256], f32)

                m_ts = []
                for q in range(8):
                    j = j0 + q
                    m_t = m_pool.tile([NSLOT, 256], emb_dt)
                    if M_PATTERN[j % len(M_PATTERN)] == "g":
                        nc.gpsimd.local_scatter(
                            m_t[:], ones2[:], scat_t[:, 2 * j:2 * j + 2],
                            channels=NSLOT, num_elems=256, num_idxs=2)
                    else:
                        nc.vector.tensor_scalar(
                            m_t[:], iota_t[:], idxc_t[:, j:j + 1], None,
                            mybir.AluOpType.is_equal)
                    m_ts.append(m_t)

                pn = pn_psum.tile([NSLOT, 1024], f32, space="PSUM")
                for q in range(8):
                    jc = (j0 + q) % CHUNK_T
                    nc.tensor.matmul(
                        pn[:, q * NSLOT:(q + 1) * NSLOT],
                        lhsT=xc[:, jc * NSLOT:(jc + 1) * NSLOT],
                        rhs=w24_t[:], start=True, stop=True)
                emb_h = emb_pool.tile([NSLOT, 1024], emb_dt, tag="embh")
                nc.scalar.activation(
                    emb_h[:], pn[:], mybir.ActivationFunctionType.Relu)
                if GRID_MODE == "hilo":
                    relu32 = emb_pool.tile([NSLOT, 1024], f32, tag="relu32")
                    nc.scalar.activation(
                        relu32[:], pn[:], mybir.ActivationFunctionType.Relu)
                    emb_l = emb_pool.tile([NSLOT, 1024], bf16, tag="embl")
                    nc.vector.tensor_tensor(
                        emb_l[:], relu32[:], emb_h[:],
                        mybir.AluOpType.subtract)

                for q in range(8):
                    j = j0 + q
                    m_t = m_ts[q]

                    if j % 2 == 0:
                        gr = gr_psum.tile([NSLOT, 512], f32, space="PSUM")
                    dst = gr[:, (j % 2) * 256:(j % 2) * 256 + 256]
                    sl = slice(q * NSLOT, (q + 1) * NSLOT)
                    if GRID_MODE == "hilo":
                        nc.tensor.matmul(dst, lhsT=emb_h[:, sl], rhs=m_t[:],
                                         start=True, stop=False)
                        nc.tensor.matmul(dst, lhsT=emb_l[:, sl], rhs=m_t[:],
                                         start=False, stop=True)
                    else:
                        nc.tensor.matmul(dst, lhsT=emb_h[:, sl], rhs=m_t[:],
                                         start=True, stop=True)

                    if j % 2 == 1:
                        sdst = stage[:, (j % FLUSH_T - 1) * 256:
                                     (j % FLUSH_T + 1) * 256]
                        pat = COPY_PATTERN[(j // 2) % len(COPY_PATTERN)]
                        if pat == "v":
                            nc.vector.tensor_copy(sdst, gr[:])
                        else:
                            nc.scalar.copy(sdst, gr[:])

                if j0 % FLUSH_T == FLUSH_T - 8:
                    fl = j0 // FLUSH_T
                    nc.sync.dma_start(
                        grid[:, fl * FLUSH_T * 256:(fl + 1) * FLUSH_T * 256],
                        stage[:])

    nc.compile()
    return nc


def _get_nc():
    if "nc" not in _cached:
        _cached["nc"] = _build_kernel()
    return _cached["nc"]


def _split_bf16(a):
    hi = a.astype(BF16)
    lo = (a - hi.astype(np.float32)).astype(BF16)
    return hi, lo


# ---------------------------------------------------------------- host prep
def _fold_bn(W, b, bn_gamma, bn_beta, bn_mean, bn_var):
    s = (bn_gamma / np.sqrt(bn_var + np.float32(BN_EPS))).astype(np.float32)
    Wp = (W * s[:, None]).T.astype(np.float32)            # [3, 64]
    bp = ((b - bn_mean) * s + bn_beta).astype(np.float32)  # [64]
    w8 = np.zeros((8, NSLOT), np.float32)
    w8[0:3, 0:D] = Wp
    w8[3, 0:D] = bp
    w8[4:7, D:2 * D] = Wp
    w8[7, D:2 * D] = bp
    wh, wl = _split_bf16(w8)
    return np.concatenate([wh, wh, wl], axis=0)   # [24, 128]


def _prep_core(pcl, mask, idx, half):
    """Pack one core's points into the task layout."""
    lo_cell = half * HALF_CELLS
    idx = idx.astype(np.int64)
    keep = mask & (idx >= lo_cell) & (idx < lo_cell + HALF_CELLS)
    il = idx[keep] - lo_cell
    pts = pcl[keep].astype(np.float32)

    # task j owns cells [256j, 256j+256) (A) and [102400+256j, +256) (B)
    tid = (il % (HALF_CELLS // 2)) >> 8
    order = np.argsort(tid, kind="stable")
    il = il[order]
    pts = pts[order]
    tid = tid[order]
    cloc = (il & 255)                        # local cell within 256-window
    rowbase = (il >= (HALF_CELLS // 2)) * 4  # 0 for half A, 4 for half B
    counts = np.bincount(tid, minlength=T)
    if counts.max() > NSLOT:
        raise RuntimeError(
            f"task overflow: {counts.max()} points in one 512-cell window")
    starts = np.zeros(T + 1, np.int64)
    np.cumsum(counts, out=starts[1:])
    slot = np.arange(len(il)) - starts[tid]
    col = tid * NSLOT + slot

    xt = np.zeros((8, T * NSLOT), np.float32)
    xt[rowbase, col] = pts[:, 0]
    xt[rowbase + 1, col] = pts[:, 1]
    xt[rowbase + 2, col] = pts[:, 2]
    xt[rowbase + 3, col] = 1.0
    scat = np.full((NSLOT, 2 * T), -1, np.int16)
    scat[slot, 2 * tid] = cloc.astype(np.int16)
    idxcol = np.full((NSLOT, T), -1.0, np.float32)
    idxcol[slot, tid] = cloc.astype(np.float32)
    xh, xl = _split_bf16(xt)
    xt24 = np.concatenate([xh, xl, xh], axis=0)   # [24, T*128]
    return xt24, scat, idxcol


def make_in_maps(previous_pcl, previous_mask, previous_grid,
                 current_pcl, current_mask, current_grid,
                 W, b, bn_gamma, bn_beta, bn_mean, bn_var):
    w24 = _fold_bn(np.asarray(W), np.asarray(b), np.asarray(bn_gamma),
                   np.asarray(bn_beta), np.asarray(bn_mean),
                   np.asarray(bn_var))
    ml_f16 = np.float16 if GRID_MODE == "f16" else BF16
    iota = np.tile(np.arange(256, dtype=np.float32), (NSLOT, 1)).astype(ml_f16)
    frames = [
        (np.asarray(previous_pcl), np.asarray(previous_mask),
         np.asarray(previous_grid)),
        (np.asarray(current_pcl), np.asarray(current_mask),
         np.asarray(current_grid)),
    ]
    in_maps = []
    for core in range(N_CORES):
        q = core // 2          # pair: q = 2*b + frame
        bb, fr = q // 2, q % 2
        pcl, mask, gidx = frames[fr]
        xt24, scat, idxcol = _prep_core(pcl[bb], np.asarray(mask[bb], bool),
                                        gidx[bb], core % 2)
        in_maps.append({"xt24": xt24, "scat": scat, "idxc": idxcol,
                        "w24": w24, "iota": iota})
    return in_maps


def assemble_output(results):
    qh = HALF_CELLS // 2
    out = np.empty((B * 2, D, P_CELLS), np.float32)
    for q in range(B * 2):
        for h in range(2):
            dev = results[2 * q + h]["grid"]       # [128, 102400]
            lo = h * HALF_CELLS
            out[q, :, lo:lo + qh] = dev[:D]
            out[q, :, lo + qh:lo + HALF_CELLS] = dev[D:]
    return out.reshape(B * 2, D, N_PX, N_PY)


# ---------------------------------------------------------------- entry point
def kernel(previous_pcl, previous_mask, previous_grid,
           current_pcl, current_mask, current_grid,
           W, b, bn_gamma, bn_beta, bn_mean, bn_var,
           _trace=False, _trace_cores=None):
    from concourse.bass_utils import run_bass_kernel_spmd

    nc = _get_nc()
    in_maps = make_in_maps(previous_pcl, previous_mask, previous_grid,
                           current_pcl, current_mask, current_grid,
                           W, b, bn_gamma, bn_beta, bn_mean, bn_var)
    res = run_bass_kernel_spmd(nc, in_maps, core_ids=list(range(N_CORES)),
                               trace=_trace, trace_cores=_trace_cores)
    out = assemble_output(res.results)
    if _trace:
        _cached["last_result"] = res
    return out


# revision 24
# speedup vs baseline: 1.0914x; 1.0914x over previous
"""Trainium2 Bass kernel for nn_Encoder_71313636983306 (pillar scatter encoder).

Computes, for each (batch, frame) pair:
    emb = relu(BN(Linear(pcl))) * mask          # [N, 64] point embeddings
    grid = scatter_add(emb, cell_idx)           # [64, 640*640]
and returns the 4 grids stacked as [B*2, 64, 640, 640] (f32).

Sharding: 8 cores = 4 (batch, frame) pairs x 2 grid halves. Each core
processes the (unmasked) points of its pair that land in its half of the
640x640 grid and writes a dense [64, 204800] f32 half-grid.

Device algorithm (per core): the half-grid is covered by T tasks; task j
owns cells [Wh*j, Wh*j+Wh) ("A") and [102400 + Wh*j, +Wh) ("B"), Wh=WIN/2.
The host packs each task's points (<=128, checked) into 128 "slots". All PE
matmuls are bf16 (full fp32 PE passes are 4x slower and fp32 weight loads
get no fast-weight-load); precision comes from hi/lo splits. Per task:
  1. pointnet: ONE bf16 matmul with K=24: [xh; xl; xh] stacked against
     [Wh; Wh; Wl] computes xh@Wh + xl@Wh + xh@Wl (residual ~2^-18) ->
     PSUM [128slots, 128]: emb placed in columns 0:64 for A-points / 64:128
     for B-points, bias folded in via a constant-1 coordinate row.
  2. relu: ScalarE PSUM->SBUF twice (bf16 "hi" + f32), VectorE subtract
     gives the bf16 "lo" residual (once per 4-task quad on [128, 512]).
  3. one-hot M[128slots, Wh] bf16: GPSIMD local_scatter (int16 indices) or
     DVE is_equal(iota, idx), alternating per M_PATTERN.
  4. grid matmul emb^T @ M as two accumulating bf16 matmuls (hi + lo)
     -> PSUM [128, Wh] = the task's WIN output cells.
  5. copy PSUM -> SBUF staging (DVE/ACT per COPY_PATTERN); every FLUSH_T
     tasks one >=1.3 MB DMA writes the staging buffer to HBM.
"""
import numpy as np
import ml_dtypes

BF16 = ml_dtypes.bfloat16

# ---------------------------------------------------------------- constants
B = 2
D = 64
N_PX = N_PY = 640
P_CELLS = N_PX * N_PY          # 409600
HALF_CELLS = P_CELLS // 2      # 204800 cells per core
QH = HALF_CELLS // 2           # 102400: A/B half-of-half offset
NSLOT = 128                    # point slots per task
BN_EPS = 1e-5
N_CORES = 8

GRID_MODE = "hilo"             # "hilo": bf16 hi+lo grid matmuls; "f16": single f16
COPY_PATTERN = "vs"            # stage-copy engines by task (v=DVE, s=ACT)
M_PATTERN = "vg"               # one-hot build engines by task (v=DVE, g=GPSIMD)
WIN_LIST = (640, 512)          # preferred window; falls back on task overflow

# per-WIN derived loop constants: tasks, xt-chunk tasks, flush tasks
_DERIVED = {640: dict(T=320, CHUNK_T=40, FLUSH_T=8),
            512: dict(T=400, CHUNK_T=40, FLUSH_T=16)}

_cached = {}


# ---------------------------------------------------------------- device code
def _build_kernel(win):
    from contextlib import ExitStack
    import concourse.tile as tile
    from concourse import bacc, mybir

    f32 = mybir.dt.float32
    bf16 = mybir.dt.bfloat16
    i16 = mybir.dt.int16
    f16 = mybir.dt.float16
    emb_dt = f16 if GRID_MODE == "f16" else bf16

    cfg = _DERIVED[win]
    T, CHUNK_T, FLUSH_T = cfg["T"], cfg["CHUNK_T"], cfg["FLUSH_T"]
    WH = win // 2

    nc = bacc.Bacc("TRN2", target_bir_lowering=False, debug=False,
                   num_devices=N_CORES)

    xt24 = nc.dram_tensor("xt24", [24, T * NSLOT], bf16,
                          kind="ExternalInput").ap()
    scat = nc.dram_tensor("scat", [NSLOT, 2 * T], i16,
                          kind="ExternalInput").ap()
    idxc = nc.dram_tensor("idxc", [NSLOT, T], f32, kind="ExternalInput").ap()
    w24 = nc.dram_tensor("w24", [24, NSLOT], bf16, kind="ExternalInput").ap()
    iota = nc.dram_tensor("iota", [NSLOT, WH], f32, kind="ExternalInput").ap()
    # Output keeps the staging layout: row p = 64*h + d holds cells
    # [102400*h + WH*j, +WH) of task j; the host deinterleaves the halves.
    grid = nc.dram_tensor("grid", [2 * D, T * WH], f32,
                          kind="ExternalOutput").ap()

    with tile.TileContext(nc) as tc:
        with ExitStack() as ctx:
            consts = ctx.enter_context(tc.tile_pool(name="consts", bufs=1))
            xt_pool = ctx.enter_context(tc.tile_pool(name="xtc", bufs=3))
            emb_pool = ctx.enter_context(tc.tile_pool(name="emb", bufs=4))
            m_pool = ctx.enter_context(tc.tile_pool(name="m", bufs=12))
            stage_pool = ctx.enter_context(tc.tile_pool(name="stage", bufs=3))
            pn_psum = ctx.enter_context(
                tc.tile_pool(name="pnps", bufs=3, space="PSUM"))
            gr_psum = ctx.enter_context(
                tc.tile_pool(name="grps", bufs=5, space="PSUM"))

            w24_t = consts.tile([24, NSLOT], bf16)
            nc.sync.dma_start(w24_t[:], w24[:])
            scat_t = consts.tile([NSLOT, 2 * T], i16)
            nc.sync.dma_start(scat_t[:], scat[:])
            idxc_t = consts.tile([NSLOT, T], f32)
            nc.sync.dma_start(idxc_t[:], idxc[:])
            iota_t = consts.tile([NSLOT, WH], f32)
            nc.sync.dma_start(iota_t[:], iota[:])
            ones2 = consts.tile([NSLOT, 2], emb_dt)
            nc.gpsimd.memset(ones2[:], 1.0)

            xc = None
            stage = None
            for g in range(T // 4):            # quad of 4 tasks
                j0 = 4 * g
                if j0 % CHUNK_T == 0:
                    xc = xt_pool.tile([24, CHUNK_T * NSLOT], bf16)
                    nc.sync.dma_start(
                        xc[:], xt24[:, j0 * NSLOT:(j0 + CHUNK_T) * NSLOT])
                if j0 % FLUSH_T == 0:
                    stage = stage_pool.tile([NSLOT, FLUSH_T * WH], f32)

                m_ts = []
                for q in range(4):
                    j = j0 + q
                    m_t = m_pool.tile([NSLOT, WH], emb_dt)
                    if M_PATTERN[j % len(M_PATTERN)] == "g":
                        nc.gpsimd.local_scatter(
                            m_t[:], ones2[:], scat_t[:, 2 * j:2 * j + 2],
                            channels=NSLOT, num_elems=WH, num_idxs=2)
                    else:
                        nc.vector.tensor_scalar(
                            m_t[:], iota_t[:], idxc_t[:, j:j + 1], None,
                            mybir.AluOpType.is_equal)
                    m_ts.append(m_t)

                pn = pn_psum.tile([NSLOT, 512], f32, space="PSUM")
                for q in range(4):
                    jc = (j0 + q) % CHUNK_T
                    nc.tensor.matmul(
                        pn[:, q * NSLOT:(q + 1) * NSLOT],
                        lhsT=xc[:, jc * NSLOT:(jc + 1) * NSLOT],
                        rhs=w24_t[:], start=True, stop=True)
                emb_h = emb_pool.tile([NSLOT, 512], emb_dt, tag="embh")
                nc.scalar.activation(
                    emb_h[:], pn[:], mybir.ActivationFunctionType.Relu)
                if GRID_MODE == "hilo":
                    relu32 = emb_pool.tile([NSLOT, 512], f32, tag="relu32")
                    nc.scalar.activation(
                        relu32[:], pn[:], mybir.ActivationFunctionType.Relu)
                    emb_l = emb_pool.tile([NSLOT, 512], bf16, tag="embl")
                    nc.vector.tensor_tensor(
                        emb_l[:], relu32[:], emb_h[:],
                        mybir.AluOpType.subtract)

                for q in range(4):
                    j = j0 + q
                    m_t = m_ts[q]
                    gr = gr_psum.tile([NSLOT, WH], f32, space="PSUM")
                    sl = slice(q * NSLOT, (q + 1) * NSLOT)
                    if GRID_MODE == "hilo":
                        nc.tensor.matmul(gr[:], lhsT=emb_h[:, sl], rhs=m_t[:],
                                         start=True, stop=False)
                        nc.tensor.matmul(gr[:], lhsT=emb_l[:, sl], rhs=m_t[:],
                                         start=False, stop=True)
                    else:
                        nc.tensor.matmul(gr[:], lhsT=emb_h[:, sl], rhs=m_t[:],
                                         start=True, stop=True)

                    sdst = stage[:, (j % FLUSH_T) * WH:(j % FLUSH_T + 1) * WH]
                    if COPY_PATTERN[j % len(COPY_PATTERN)] == "v":
                        nc.vector.tensor_copy(sdst, gr[:])
                    else:
                        nc.scalar.copy(sdst, gr[:])

                if j0 % FLUSH_T == FLUSH_T - 4:
                    fl = j0 // FLUSH_T
                    nc.sync.dma_start(
                        grid[:, fl * FLUSH_T * WH:(fl + 1) * FLUSH_T * WH],
                        stage[:])

    nc.compile()
    return nc


def _get_nc(win):
    key = ("nc", win, GRID_MODE, M_PATTERN, COPY_PATTERN)
    if key not in _cached:
        _cached[key] = _build_kernel(win)
    return _cached[key]


def _split_bf16(a):
    hi = a.astype(BF16)
    lo = (a - hi.astype(np.float32)).astype(BF16)
    return hi, lo


class _TaskOverflow(RuntimeError):
    pass


# ---------------------------------------------------------------- host prep
def _fold_bn(W, b, bn_gamma, bn_beta, bn_mean, bn_var):
    s = (bn_gamma / np.sqrt(bn_var + np.float32(BN_EPS))).astype(np.float32)
    Wp = (W * s[:, None]).T.astype(np.float32)            # [3, 64]
    bp = ((b - bn_mean) * s + bn_beta).astype(np.float32)  # [64]
    w8 = np.zeros((8, NSLOT), np.float32)
    w8[0:3, 0:D] = Wp
    w8[3, 0:D] = bp
    w8[4:7, D:2 * D] = Wp
    w8[7, D:2 * D] = bp
    wh, wl = _split_bf16(w8)
    return np.concatenate([wh, wh, wl], axis=0)   # [24, 128]


def _prep_core(pcl, mask, idx, half, win):
    """Pack one core's points into the task layout. Raises on task overflow."""
    T = _DERIVED[win]["T"]
    WH = win // 2
    lo_cell = half * HALF_CELLS
    idx = idx.astype(np.int64)
    keep = mask & (idx >= lo_cell) & (idx < lo_cell + HALF_CELLS)
    il = idx[keep] - lo_cell
    pts = pcl[keep].astype(np.float32)

    # task j owns cells [WH*j, +WH) (A) and [102400 + WH*j, +WH) (B)
    tid = (il % QH) // WH
    order = np.argsort(tid, kind="stable")
    il = il[order]
    pts = pts[order]
    tid = tid[order]
    cloc = (il % QH) - tid * WH              # local cell within WH-window
    rowbase = (il >= QH) * 4                 # 0 for half A, 4 for half B
    counts = np.bincount(tid, minlength=T)
    if counts.max() > NSLOT:
        raise _TaskOverflow(
            f"{counts.max()} points in one {win}-cell window")
    starts = np.zeros(T + 1, np.int64)
    np.cumsum(counts, out=starts[1:])
    slot = np.arange(len(il)) - starts[tid]
    col = tid * NSLOT + slot

    xt = np.zeros((8, T * NSLOT), np.float32)
    xt[rowbase, col] = pts[:, 0]
    xt[rowbase + 1, col] = pts[:, 1]
    xt[rowbase + 2, col] = pts[:, 2]
    xt[rowbase + 3, col] = 1.0
    scat = np.full((NSLOT, 2 * T), -1, np.int16)
    scat[slot, 2 * tid] = cloc.astype(np.int16)
    idxcol = np.full((NSLOT, T), -1.0, np.float32)
    idxcol[slot, tid] = cloc.astype(np.float32)
    xh, xl = _split_bf16(xt)
    xt24 = np.concatenate([xh, xl, xh], axis=0)   # [24, T*128]
    return xt24, scat, idxcol


def make_in_maps(win, previous_pcl, previous_mask, previous_grid,
                 current_pcl, current_mask, current_grid,
                 W, b, bn_gamma, bn_beta, bn_mean, bn_var):
    w24 = _fold_bn(np.asarray(W), np.asarray(b), np.asarray(bn_gamma),
                   np.asarray(bn_beta), np.asarray(bn_mean),
                   np.asarray(bn_var))
    iota = np.tile(np.arange(win // 2, dtype=np.float32), (NSLOT, 1))
    frames = [
        (np.asarray(previous_pcl), np.asarray(previous_mask),
         np.asarray(previous_grid)),
        (np.asarray(current_pcl), np.asarray(current_mask),
         np.asarray(current_grid)),
    ]
    in_maps = []
    for core in range(N_CORES):
        q = core // 2          # pair: q = 2*b + frame
        bb, fr = q // 2, q % 2
        pcl, mask, gidx = frames[fr]
        xt24, scat, idxcol = _prep_core(pcl[bb], np.asarray(mask[bb], bool),
                                        gidx[bb], core % 2, win)
        in_maps.append({"xt24": xt24, "scat": scat, "idxc": idxcol,
                        "w24": w24, "iota": iota})
    return in_maps


def assemble_output(results):
    out = np.empty((B * 2, D, P_CELLS), np.float32)
    for q in range(B * 2):
        for h in range(2):
            dev = results[2 * q + h]["grid"]       # [128, 102400]
            lo = h * HALF_CELLS
            out[q, :, lo:lo + QH] = dev[:D]
            out[q, :, lo + QH:lo + HALF_CELLS] = dev[D:]
    return out.reshape(B * 2, D, N_PX, N_PY)


# ---------------------------------------------------------------- entry point
def kernel(previous_pcl, previous_mask, previous_grid,
           current_pcl, current_mask, current_grid,
           W, b, bn_gamma, bn_beta, bn_mean, bn_var,
           _trace=False, _trace_cores=None):
    from concourse.bass_utils import run_bass_kernel_spmd

    kw = dict(previous_pcl=previous_pcl, previous_mask=previous_mask,
              previous_grid=previous_grid, current_pcl=current_pcl,
              current_mask=current_mask, current_grid=current_grid,
              W=W, b=b, bn_gamma=bn_gamma, bn_beta=bn_beta,
              bn_mean=bn_mean, bn_var=bn_var)
    in_maps = None
    win = WIN_LIST[-1]
    for win in WIN_LIST:
        try:
            in_maps = make_in_maps(win, **kw)
            break
        except _TaskOverflow:
            if win == WIN_LIST[-1]:
                raise
    nc = _get_nc(win)
    res = run_bass_kernel_spmd(nc, in_maps, core_ids=list(range(N_CORES)),
                               trace=_trace, trace_cores=_trace_cores)
    out = assemble_output(res.results)
    if _trace:
        _cached["last_result"] = res
    return out
